# revision 1
# baseline (speedup 1.0000x reference)
import os

os.environ.setdefault("NEURON_CC_FLAGS", "--auto-cast=none")

import numpy as np
import jax
import jax.numpy as jnp

# Model dims (hardcoded per spec)
B, L, N, K, D = 2048, 16, 4, 8, 256
NCORES = 8
NHEAD = 4
NOISE_W = 1.0
_C = NOISE_W + 1e-6

f32 = jnp.float32


# ---------- complex helpers on (re, im) pairs ----------
def cmm(ar, ai, br, bi):
    # complex matmul over last two dims
    return ar @ br - ai @ bi, ar @ bi + ai @ br


def chconj(ar, ai):
    # conjugate transpose of last two dims
    return jnp.swapaxes(ar, -1, -2), -jnp.swapaxes(ai, -1, -2)


def cinv2(ar, ai):
    # inverse of (..., 2, 2) complex matrix, closed form adj/det
    a_r, a_i = ar[..., 0, 0], ai[..., 0, 0]
    b_r, b_i = ar[..., 0, 1], ai[..., 0, 1]
    c_r, c_i = ar[..., 1, 0], ai[..., 1, 0]
    d_r, d_i = ar[..., 1, 1], ai[..., 1, 1]
    det_r = a_r * d_r - a_i * d_i - (b_r * c_r - b_i * c_i)
    det_i = a_r * d_i + a_i * d_r - (b_r * c_i + b_i * c_r)
    idn = 1.0 / (det_r * det_r + det_i * det_i)
    e_r, e_i = det_r * idn, -det_i * idn  # 1/det

    def mul(x_r, x_i):
        return x_r * e_r - x_i * e_i, x_r * e_i + x_i * e_r

    i00 = mul(d_r, d_i)
    i01 = mul(-b_r, -b_i)
    i10 = mul(-c_r, -c_i)
    i11 = mul(a_r, a_i)
    inv_r = jnp.stack(
        [jnp.stack([i00[0], i01[0]], -1), jnp.stack([i10[0], i11[0]], -1)], -2
    )
    inv_i = jnp.stack(
        [jnp.stack([i00[1], i01[1]], -1), jnp.stack([i10[1], i11[1]], -1)], -2
    )
    return inv_r, inv_i


def cinv(ar, ai):
    # recursive 2x2-block (Schur) inverse for (..., n, n) complex, n power of 2
    n = ar.shape[-1]
    if n == 2:
        return cinv2(ar, ai)
    h = n // 2
    Pr, Pi_ = ar[..., :h, :h], ai[..., :h, :h]
    Qr, Qi = ar[..., :h, h:], ai[..., :h, h:]
    Cr, Ci = ar[..., h:, :h], ai[..., h:, :h]
    Rr, Ri = ar[..., h:, h:], ai[..., h:, h:]
    Pinv = cinv(Pr, Pi_)
    Tr, Ti = cmm(Pinv[0], Pinv[1], Qr, Qi)  # P^-1 Q
    CT = cmm(Cr, Ci, Tr, Ti)
    Sr, Si_ = Rr - CT[0], Ri - CT[1]  # Schur complement
    Sinv = cinv(Sr, Si_)
    CP = cmm(Cr, Ci, Pinv[0], Pinv[1])  # C P^-1
    Ur, Ui = cmm(Tr, Ti, Sinv[0], Sinv[1])  # P^-1 Q S^-1
    BLr, BLi = cmm(Sinv[0], Sinv[1], CP[0], CP[1])  # S^-1 C P^-1
    TLr = Pinv[0] + cmm(Ur, Ui, CP[0], CP[1])[0]
    TLi = Pinv[1] + cmm(Ur, Ui, CP[0], CP[1])[1]
    top_r = jnp.concatenate([TLr, -Ur], -1)
    top_i = jnp.concatenate([TLi, -Ui], -1)
    bot_r = jnp.concatenate([-BLr, Sinv[0]], -1)
    bot_i = jnp.concatenate([-BLi, Sinv[1]], -1)
    return jnp.concatenate([top_r, bot_r], -2), jnp.concatenate([top_i, bot_i], -2)


# ---------- model pieces (f32) ----------
def mlp2(p, x):
    h = jax.nn.relu(x @ p["l1"]["W"] + p["l1"]["b"])
    return h @ p["l2"]["W"] + p["l2"]["b"]


def layer_norm(x, g, b, eps=1e-5):
    m = x.mean(-1, keepdims=True)
    v = ((x - m) ** 2).mean(-1, keepdims=True)
    return (x - m) / jnp.sqrt(v + eps) * g + b


def per_ap_mmse(H_q, y_q):
    Hr, Hi = H_q[..., 0], H_q[..., 1]  # (b,L,N,K)
    yr, yi = y_q[..., 0], y_q[..., 1]  # (b,L,N)
    HHr, HHi = chconj(Hr, Hi)  # (b,L,K,N)
    Ryr, Ryi = cmm(Hr, Hi, HHr, HHi)  # (b,L,N,N)
    eye = jnp.eye(N, dtype=f32) * _C
    Ryr = Ryr + eye
    Rinv = cinv(Ryr, Ryi)
    # u = Ry^-1 y  (matvec)
    ur = Rinv[0] @ yr[..., None] - Rinv[1] @ yi[..., None]
    ui = Rinv[0] @ yi[..., None] + Rinv[1] @ yr[..., None]
    # s = H^H u
    sr = HHr @ ur - HHi @ ui
    si = HHr @ ui + HHi @ ur
    return jnp.concatenate([sr, si], -1)  # (b,L,K,2)


def global_mmse(H_q, y_q):
    b = H_q.shape[0]
    Hr = H_q[..., 0].reshape(b, L * N, K)
    Hi = H_q[..., 1].reshape(b, L * N, K)
    yr = y_q[..., 0].reshape(b, L * N)
    yi = y_q[..., 1].reshape(b, L * N)
    HHr, HHi = chconj(Hr, Hi)  # (b,K,LN)
    Rr, Ri = cmm(HHr, HHi, Hr, Hi)  # (b,K,K)
    Rr = Rr + jnp.eye(K, dtype=f32) * _C
    Rinv = cinv(Rr, Ri)
    hyr = HHr @ yr[..., None] - HHi @ yi[..., None]
    hyi = HHr @ yi[..., None] + HHi @ yr[..., None]
    sr = Rinv[0] @ hyr - Rinv[1] @ hyi
    si = Rinv[0] @ hyi + Rinv[1] @ hyr
    return jnp.concatenate([sr, si], -1)  # (b,K,2)


def mha(p, x):
    b, s, d = x.shape
    dh = d // NHEAD
    q = (x @ p["Wq"] + p["bq"]).reshape(b, s, NHEAD, dh)
    k = (x @ p["Wk"] + p["bk"]).reshape(b, s, NHEAD, dh)
    v = (x @ p["Wv"] + p["bv"]).reshape(b, s, NHEAD, dh)
    a = jax.nn.softmax(
        jnp.einsum("bqhd,bkhd->bhqk", q, k) / np.sqrt(dh).astype(np.float32), axis=-1
    )
    o = jnp.einsum("bhqk,bkhd->bqhd", a, v).reshape(b, s, d)
    return o @ p["Wo"] + p["bo"]


def tf_layer(p, x):
    x = x + mha(p, layer_norm(x, p["ln1g"], p["ln1b"]))
    h = layer_norm(x, p["ln2g"], p["ln2b"])
    return x + (jax.nn.relu(h @ p["Wf1"] + p["bf1"]) @ p["Wf2"] + p["bf2"])


def _forward(s_hat_q, H_q, y_q, bitwidth_features, local_snr, params):
    s_global = global_mmse(H_q, y_q)  # (b,K,2)
    s_init = per_ap_mmse(H_q, y_q)  # (b,L,K,2)
    f_init = mlp2(params["mmse_init"], s_init)
    f_demod = mlp2(params["demod"], s_hat_q)
    sg_b = jnp.broadcast_to(s_global[:, None], s_init.shape)
    f_glob = mlp2(params["gmmse"], sg_b)
    b = H_q.shape[0]
    H_flat = jnp.transpose(H_q, (0, 1, 3, 2, 4)).reshape(b, L, K, N * 2)
    f_chan = mlp2(params["chan"], H_flat)
    h_power = (H_q**2).sum(axis=(2, 4))  # (b,L,K)
    interference = h_power.sum(axis=2, keepdims=True) - h_power
    ifeat = jnp.stack([jnp.log1p(h_power), jnp.log1p(interference)], -1)
    combined = jnp.concatenate(
        [f_init, f_demod, f_glob, f_chan, bitwidth_features, ifeat, local_snr], -1
    )
    node = mlp2(params["fusion"], combined)  # (b,L,K,D)
    flat = node.reshape(-1, D)
    # global BatchNorm stats across all cores (training-mode batch stats)
    mu_loc = flat.mean(0)
    m2_loc = (flat * flat).mean(0)
    mu = jax.lax.pmean(mu_loc, axis_name="x")
    m2 = jax.lax.pmean(m2_loc, axis_name="x")
    var = m2 - mu * mu
    flat = (flat - mu) / jnp.sqrt(var + 1e-5) * params["bn"]["g"] + params["bn"]["b"]
    h = flat.reshape(node.shape)
    for gp in params["gnn"]:
        msg = mlp2(gp["msg"], h)
        mean_msg = jnp.broadcast_to(msg.mean(axis=1, keepdims=True), h.shape)
        upd = mlp2(gp["upd"], jnp.concatenate([h, mean_msg], -1))
        h = layer_norm(h + upd, gp["lng"], gp["lnb"])
    attn = jax.nn.softmax(mlp2(params["agg"], h), axis=1)
    user = (h * attn).sum(axis=1)  # (b,K,D)
    for tp in params["tf"]:
        user = tf_layer(tp, user)
    residual = user @ params["out"]["W"] + params["out"]["b"]
    detected = s_global + params["alpha"] * residual
    return detected, s_global


_pmapped = None


def _get_pmapped():
    global _pmapped
    if _pmapped is None:
        _pmapped = jax.pmap(
            _forward, axis_name="x", in_axes=(0, 0, 0, 0, 0, None)
        )
    return _pmapped


def _to_f32(x):
    return jnp.asarray(np.asarray(x), dtype=f32)


def kernel(s_hat_q, H_q, y_q, bitwidth_features, local_snr, params):
    bs = B // NCORES
    sh = _to_f32(s_hat_q).reshape(NCORES, bs, L, K, 2)
    hq = _to_f32(H_q).reshape(NCORES, bs, L, N, K, 2)
    yq = _to_f32(y_q).reshape(NCORES, bs, L, N, 2)
    bw = _to_f32(bitwidth_features).reshape(NCORES, bs, L, K, 3)
    snr = _to_f32(local_snr).reshape(NCORES, bs, L, K, 1)
    p32 = jax.tree_util.tree_map(_to_f32, params)
    detected, s_global = _get_pmapped()(sh, hq, yq, bw, snr, p32)
    detected = np.asarray(detected, dtype=np.float32).reshape(B, K, 2)
    s_global = np.asarray(s_global, dtype=np.float32).reshape(B, K, 2)
    return detected, s_global


# revision 4
# speedup vs baseline: 1.1849x; 1.1849x over previous
import os

os.environ.setdefault("NEURON_CC_FLAGS", "--auto-cast=none")

import numpy as np
import jax
import jax.numpy as jnp

try:
    jax.config.update("jax_compilation_cache_dir", "/root/.cache/jax_cc_cache")
    jax.config.update("jax_persistent_cache_min_compile_time_secs", 5.0)
except Exception:
    pass

# Model dims (hardcoded per spec)
B, L, N, K, D = 2048, 16, 4, 8, 256
NCORES = 8
BS = B // NCORES
NHEAD = 4
NOISE_W = 1.0
_C = NOISE_W + 1e-6

f32 = jnp.float32

# packed input blob layout (per device, f32 counts)
_SIZES = [
    BS * L * K * 2,      # s_hat_q
    BS * L * N * K * 2,  # H_q
    BS * L * N * 2,      # y_q
    BS * L * K * 3,      # bitwidth_features
    BS * L * K * 1,      # local_snr
]
_OFFS = np.concatenate([[0], np.cumsum(_SIZES)]).tolist()


# ---------- complex helpers on (re, im) pairs ----------
def cmm(ar, ai, br, bi):
    return ar @ br - ai @ bi, ar @ bi + ai @ br


def chconj(ar, ai):
    return jnp.swapaxes(ar, -1, -2), -jnp.swapaxes(ai, -1, -2)


def cinv2(ar, ai):
    a_r, a_i = ar[..., 0, 0], ai[..., 0, 0]
    b_r, b_i = ar[..., 0, 1], ai[..., 0, 1]
    c_r, c_i = ar[..., 1, 0], ai[..., 1, 0]
    d_r, d_i = ar[..., 1, 1], ai[..., 1, 1]
    det_r = a_r * d_r - a_i * d_i - (b_r * c_r - b_i * c_i)
    det_i = a_r * d_i + a_i * d_r - (b_r * c_i + b_i * c_r)
    idn = 1.0 / (det_r * det_r + det_i * det_i)
    e_r, e_i = det_r * idn, -det_i * idn

    def mul(x_r, x_i):
        return x_r * e_r - x_i * e_i, x_r * e_i + x_i * e_r

    i00 = mul(d_r, d_i)
    i01 = mul(-b_r, -b_i)
    i10 = mul(-c_r, -c_i)
    i11 = mul(a_r, a_i)
    inv_r = jnp.stack(
        [jnp.stack([i00[0], i01[0]], -1), jnp.stack([i10[0], i11[0]], -1)], -2
    )
    inv_i = jnp.stack(
        [jnp.stack([i00[1], i01[1]], -1), jnp.stack([i10[1], i11[1]], -1)], -2
    )
    return inv_r, inv_i


def cinv(ar, ai):
    n = ar.shape[-1]
    if n == 2:
        return cinv2(ar, ai)
    h = n // 2
    Pr, Pi_ = ar[..., :h, :h], ai[..., :h, :h]
    Qr, Qi = ar[..., :h, h:], ai[..., :h, h:]
    Cr, Ci = ar[..., h:, :h], ai[..., h:, :h]
    Rr, Ri = ar[..., h:, h:], ai[..., h:, h:]
    Pinv = cinv(Pr, Pi_)
    Tr, Ti = cmm(Pinv[0], Pinv[1], Qr, Qi)
    CT = cmm(Cr, Ci, Tr, Ti)
    Sr, Si_ = Rr - CT[0], Ri - CT[1]
    Sinv = cinv(Sr, Si_)
    CP = cmm(Cr, Ci, Pinv[0], Pinv[1])
    Ur, Ui = cmm(Tr, Ti, Sinv[0], Sinv[1])
    BLr, BLi = cmm(Sinv[0], Sinv[1], CP[0], CP[1])
    UCP = cmm(Ur, Ui, CP[0], CP[1])
    TLr = Pinv[0] + UCP[0]
    TLi = Pinv[1] + UCP[1]
    top_r = jnp.concatenate([TLr, -Ur], -1)
    top_i = jnp.concatenate([TLi, -Ui], -1)
    bot_r = jnp.concatenate([-BLr, Sinv[0]], -1)
    bot_i = jnp.concatenate([-BLi, Sinv[1]], -1)
    return jnp.concatenate([top_r, bot_r], -2), jnp.concatenate([top_i, bot_i], -2)


# ---------- model pieces (f32) ----------
def mlp2(p, x):
    h = jax.nn.relu(x @ p["l1"]["W"] + p["l1"]["b"])
    return h @ p["l2"]["W"] + p["l2"]["b"]


def layer_norm(x, g, b, eps=1e-5):
    m = x.mean(-1, keepdims=True)
    v = ((x - m) ** 2).mean(-1, keepdims=True)
    return (x - m) / jnp.sqrt(v + eps) * g + b


def per_ap_mmse(H_q, y_q):
    Hr, Hi = H_q[..., 0], H_q[..., 1]
    yr, yi = y_q[..., 0], y_q[..., 1]
    HHr, HHi = chconj(Hr, Hi)
    Ryr, Ryi = cmm(Hr, Hi, HHr, HHi)
    Ryr = Ryr + jnp.eye(N, dtype=f32) * _C
    Rinv = cinv(Ryr, Ryi)
    ur = Rinv[0] @ yr[..., None] - Rinv[1] @ yi[..., None]
    ui = Rinv[0] @ yi[..., None] + Rinv[1] @ yr[..., None]
    sr = HHr @ ur - HHi @ ui
    si = HHr @ ui + HHi @ ur
    return jnp.concatenate([sr, si], -1)


def global_mmse(H_q, y_q):
    b = H_q.shape[0]
    Hr = H_q[..., 0].reshape(b, L * N, K)
    Hi = H_q[..., 1].reshape(b, L * N, K)
    yr = y_q[..., 0].reshape(b, L * N)
    yi = y_q[..., 1].reshape(b, L * N)
    HHr, HHi = chconj(Hr, Hi)
    Rr, Ri = cmm(HHr, HHi, Hr, Hi)
    Rr = Rr + jnp.eye(K, dtype=f32) * _C
    Rinv = cinv(Rr, Ri)
    hyr = HHr @ yr[..., None] - HHi @ yi[..., None]
    hyi = HHr @ yi[..., None] + HHi @ yr[..., None]
    sr = Rinv[0] @ hyr - Rinv[1] @ hyi
    si = Rinv[0] @ hyi + Rinv[1] @ hyr
    return jnp.concatenate([sr, si], -1)


def mha(p, x):
    b, s, d = x.shape
    dh = d // NHEAD
    q = (x @ p["Wq"] + p["bq"]).reshape(b, s, NHEAD, dh)
    k = (x @ p["Wk"] + p["bk"]).reshape(b, s, NHEAD, dh)
    v = (x @ p["Wv"] + p["bv"]).reshape(b, s, NHEAD, dh)
    a = jax.nn.softmax(
        jnp.einsum("bqhd,bkhd->bhqk", q, k) / np.sqrt(dh).astype(np.float32), axis=-1
    )
    o = jnp.einsum("bhqk,bkhd->bqhd", a, v).reshape(b, s, d)
    return o @ p["Wo"] + p["bo"]


def tf_layer(p, x):
    x = x + mha(p, layer_norm(x, p["ln1g"], p["ln1b"]))
    h = layer_norm(x, p["ln2g"], p["ln2b"])
    return x + (jax.nn.relu(h @ p["Wf1"] + p["bf1"]) @ p["Wf2"] + p["bf2"])


def _unpack(blob):
    o = _OFFS
    return (
        blob[o[0]:o[1]].reshape(BS, L, K, 2),
        blob[o[1]:o[2]].reshape(BS, L, N, K, 2),
        blob[o[2]:o[3]].reshape(BS, L, N, 2),
        blob[o[3]:o[4]].reshape(BS, L, K, 3),
        blob[o[4]:o[5]].reshape(BS, L, K, 1),
    )


def _forward(s_hat_q, H_q, y_q, bitwidth_features, local_snr, params):
    s_global = global_mmse(H_q, y_q)  # (bs,K,2)
    s_init = per_ap_mmse(H_q, y_q)  # (bs,L,K,2)
    f_init = mlp2(params["mmse_init"], s_init)
    f_demod = mlp2(params["demod"], s_hat_q)
    sg_b = jnp.broadcast_to(s_global[:, None], s_init.shape)
    f_glob = mlp2(params["gmmse"], sg_b)
    H_flat = jnp.transpose(H_q, (0, 1, 3, 2, 4)).reshape(BS, L, K, N * 2)
    f_chan = mlp2(params["chan"], H_flat)
    h_power = (H_q**2).sum(axis=(2, 4))
    interference = h_power.sum(axis=2, keepdims=True) - h_power
    ifeat = jnp.stack([jnp.log1p(h_power), jnp.log1p(interference)], -1)
    combined = jnp.concatenate(
        [f_init, f_demod, f_glob, f_chan, bitwidth_features, ifeat, local_snr], -1
    )
    node = mlp2(params["fusion"], combined)
    flat = node.reshape(-1, D)
    mu_loc = flat.mean(0)
    m2_loc = (flat * flat).mean(0)
    mu = jax.lax.pmean(mu_loc, axis_name="x")
    m2 = jax.lax.pmean(m2_loc, axis_name="x")
    var = m2 - mu * mu
    flat = (flat - mu) / jnp.sqrt(var + 1e-5) * params["bn"]["g"] + params["bn"]["b"]
    h = flat.reshape(node.shape)
    for gp in params["gnn"]:
        msg = mlp2(gp["msg"], h)
        mean_msg = jnp.broadcast_to(msg.mean(axis=1, keepdims=True), h.shape)
        upd = mlp2(gp["upd"], jnp.concatenate([h, mean_msg], -1))
        h = layer_norm(h + upd, gp["lng"], gp["lnb"])
    attn = jax.nn.softmax(mlp2(params["agg"], h), axis=1)
    user = (h * attn).sum(axis=1)
    for tp in params["tf"]:
        user = tf_layer(tp, user)
    residual = user @ params["out"]["W"] + params["out"]["b"]
    detected = s_global + params["alpha"] * residual
    return detected, s_global


_pmapped = None
_unpack_pm = None
_dev_params = None
_param_key = None


def kernel(s_hat_q, H_q, y_q, bitwidth_features, local_snr, params):
    global _pmapped, _unpack_pm, _dev_params, _param_key
    devices = jax.devices()[:NCORES]

    # pack all inputs into one per-device blob -> single sharded transfer
    parts = [
        np.ascontiguousarray(np.asarray(a, np.float32)).reshape(NCORES, -1)
        for a in (s_hat_q, H_q, y_q, bitwidth_features, local_snr)
    ]
    blob = np.concatenate(parts, axis=1)
    dblob = jax.device_put_sharded([blob[i] for i in range(NCORES)], devices)

    key = id(params)
    if _dev_params is None or _param_key != key:
        p32 = jax.tree_util.tree_map(
            lambda x: np.asarray(x, np.float32), params
        )
        _dev_params = jax.device_put_replicated(p32, devices)
        _param_key = key

    if _unpack_pm is None:
        _unpack_pm = jax.pmap(_unpack, in_axes=0)
    if _pmapped is None:
        _pmapped = jax.pmap(
            _forward, axis_name="x", in_axes=(0, 0, 0, 0, 0, 0)
        )

    tensors = _unpack_pm(dblob)
    detected, s_global = _pmapped(*tensors, _dev_params)
    detected = np.asarray(detected, dtype=np.float32).reshape(B, K, 2)
    s_global = np.asarray(s_global, dtype=np.float32).reshape(B, K, 2)
    return detected, s_global


# revision 5
# speedup vs baseline: 1.6122x; 1.3606x over previous
import os

os.environ.setdefault("NEURON_CC_FLAGS", "--auto-cast=none")

import numpy as np
import jax
import jax.numpy as jnp

try:
    jax.config.update("jax_compilation_cache_dir", "/root/.cache/jax_cc_cache")
    jax.config.update("jax_persistent_cache_min_compile_time_secs", 5.0)
except Exception:
    pass

# Model dims (hardcoded per spec)
B, L, N, K, D = 2048, 16, 4, 8, 256
NCORES = 8
BS = B // NCORES
NHEAD = 4
NOISE_W = 1.0
_C = NOISE_W + 1e-6

f32 = jnp.float32

# packed input blob layout (per device, f32 counts)
_SIZES = [
    BS * L * K * 2,      # s_hat_q
    BS * L * N * K * 2,  # H_q
    BS * L * N * 2,      # y_q
    BS * L * K * 3,      # bitwidth_features
    BS * L * K * 1,      # local_snr
]
_OFFS = np.concatenate([[0], np.cumsum(_SIZES)]).tolist()


# ---------- complex helpers on (re, im) pairs ----------
def cmm(ar, ai, br, bi):
    return ar @ br - ai @ bi, ar @ bi + ai @ br


def chconj(ar, ai):
    return jnp.swapaxes(ar, -1, -2), -jnp.swapaxes(ai, -1, -2)


def cinv2(ar, ai):
    a_r, a_i = ar[..., 0, 0], ai[..., 0, 0]
    b_r, b_i = ar[..., 0, 1], ai[..., 0, 1]
    c_r, c_i = ar[..., 1, 0], ai[..., 1, 0]
    d_r, d_i = ar[..., 1, 1], ai[..., 1, 1]
    det_r = a_r * d_r - a_i * d_i - (b_r * c_r - b_i * c_i)
    det_i = a_r * d_i + a_i * d_r - (b_r * c_i + b_i * c_r)
    idn = 1.0 / (det_r * det_r + det_i * det_i)
    e_r, e_i = det_r * idn, -det_i * idn

    def mul(x_r, x_i):
        return x_r * e_r - x_i * e_i, x_r * e_i + x_i * e_r

    i00 = mul(d_r, d_i)
    i01 = mul(-b_r, -b_i)
    i10 = mul(-c_r, -c_i)
    i11 = mul(a_r, a_i)
    inv_r = jnp.stack(
        [jnp.stack([i00[0], i01[0]], -1), jnp.stack([i10[0], i11[0]], -1)], -2
    )
    inv_i = jnp.stack(
        [jnp.stack([i00[1], i01[1]], -1), jnp.stack([i10[1], i11[1]], -1)], -2
    )
    return inv_r, inv_i


def cinv(ar, ai):
    n = ar.shape[-1]
    if n == 2:
        return cinv2(ar, ai)
    h = n // 2
    Pr, Pi_ = ar[..., :h, :h], ai[..., :h, :h]
    Qr, Qi = ar[..., :h, h:], ai[..., :h, h:]
    Cr, Ci = ar[..., h:, :h], ai[..., h:, :h]
    Rr, Ri = ar[..., h:, h:], ai[..., h:, h:]
    Pinv = cinv(Pr, Pi_)
    Tr, Ti = cmm(Pinv[0], Pinv[1], Qr, Qi)
    CT = cmm(Cr, Ci, Tr, Ti)
    Sr, Si_ = Rr - CT[0], Ri - CT[1]
    Sinv = cinv(Sr, Si_)
    CP = cmm(Cr, Ci, Pinv[0], Pinv[1])
    Ur, Ui = cmm(Tr, Ti, Sinv[0], Sinv[1])
    BLr, BLi = cmm(Sinv[0], Sinv[1], CP[0], CP[1])
    UCP = cmm(Ur, Ui, CP[0], CP[1])
    TLr = Pinv[0] + UCP[0]
    TLi = Pinv[1] + UCP[1]
    top_r = jnp.concatenate([TLr, -Ur], -1)
    top_i = jnp.concatenate([TLi, -Ui], -1)
    bot_r = jnp.concatenate([-BLr, Sinv[0]], -1)
    bot_i = jnp.concatenate([-BLi, Sinv[1]], -1)
    return jnp.concatenate([top_r, bot_r], -2), jnp.concatenate([top_i, bot_i], -2)


# ---------- model pieces (f32) ----------
def mlp2(p, x):
    h = jax.nn.relu(x @ p["l1"]["W"] + p["l1"]["b"])
    return h @ p["l2"]["W"] + p["l2"]["b"]


def layer_norm(x, g, b, eps=1e-5):
    m = x.mean(-1, keepdims=True)
    v = ((x - m) ** 2).mean(-1, keepdims=True)
    return (x - m) / jnp.sqrt(v + eps) * g + b


def per_ap_mmse(H_q, y_q):
    Hr, Hi = H_q[..., 0], H_q[..., 1]
    yr, yi = y_q[..., 0], y_q[..., 1]
    HHr, HHi = chconj(Hr, Hi)
    Ryr, Ryi = cmm(Hr, Hi, HHr, HHi)
    Ryr = Ryr + jnp.eye(N, dtype=f32) * _C
    Rinv = cinv(Ryr, Ryi)
    ur = Rinv[0] @ yr[..., None] - Rinv[1] @ yi[..., None]
    ui = Rinv[0] @ yi[..., None] + Rinv[1] @ yr[..., None]
    sr = HHr @ ur - HHi @ ui
    si = HHr @ ui + HHi @ ur
    return jnp.concatenate([sr, si], -1)


def global_mmse(H_q, y_q):
    b = H_q.shape[0]
    Hr = H_q[..., 0].reshape(b, L * N, K)
    Hi = H_q[..., 1].reshape(b, L * N, K)
    yr = y_q[..., 0].reshape(b, L * N)
    yi = y_q[..., 1].reshape(b, L * N)
    HHr, HHi = chconj(Hr, Hi)
    Rr, Ri = cmm(HHr, HHi, Hr, Hi)
    Rr = Rr + jnp.eye(K, dtype=f32) * _C
    Rinv = cinv(Rr, Ri)
    hyr = HHr @ yr[..., None] - HHi @ yi[..., None]
    hyi = HHr @ yi[..., None] + HHi @ yr[..., None]
    sr = Rinv[0] @ hyr - Rinv[1] @ hyi
    si = Rinv[0] @ hyi + Rinv[1] @ hyr
    return jnp.concatenate([sr, si], -1)


def mha(p, x):
    b, s, d = x.shape
    dh = d // NHEAD
    q = (x @ p["Wq"] + p["bq"]).reshape(b, s, NHEAD, dh)
    k = (x @ p["Wk"] + p["bk"]).reshape(b, s, NHEAD, dh)
    v = (x @ p["Wv"] + p["bv"]).reshape(b, s, NHEAD, dh)
    a = jax.nn.softmax(
        jnp.einsum("bqhd,bkhd->bhqk", q, k) / np.sqrt(dh).astype(np.float32), axis=-1
    )
    o = jnp.einsum("bhqk,bkhd->bqhd", a, v).reshape(b, s, d)
    return o @ p["Wo"] + p["bo"]


def tf_layer(p, x):
    x = x + mha(p, layer_norm(x, p["ln1g"], p["ln1b"]))
    h = layer_norm(x, p["ln2g"], p["ln2b"])
    return x + (jax.nn.relu(h @ p["Wf1"] + p["bf1"]) @ p["Wf2"] + p["bf2"])


def _unpack(blob):
    o = _OFFS
    return (
        blob[o[0]:o[1]].reshape(BS, L, K, 2),
        blob[o[1]:o[2]].reshape(BS, L, N, K, 2),
        blob[o[2]:o[3]].reshape(BS, L, N, 2),
        blob[o[3]:o[4]].reshape(BS, L, K, 3),
        blob[o[4]:o[5]].reshape(BS, L, K, 1),
    )


def _forward(s_hat_q, H_q, y_q, bitwidth_features, local_snr, params):
    s_global = global_mmse(H_q, y_q)  # (bs,K,2)
    s_init = per_ap_mmse(H_q, y_q)  # (bs,L,K,2)
    f_init = mlp2(params["mmse_init"], s_init)
    f_demod = mlp2(params["demod"], s_hat_q)
    sg_b = jnp.broadcast_to(s_global[:, None], s_init.shape)
    f_glob = mlp2(params["gmmse"], sg_b)
    H_flat = jnp.transpose(H_q, (0, 1, 3, 2, 4)).reshape(BS, L, K, N * 2)
    f_chan = mlp2(params["chan"], H_flat)
    h_power = (H_q**2).sum(axis=(2, 4))
    interference = h_power.sum(axis=2, keepdims=True) - h_power
    ifeat = jnp.stack([jnp.log1p(h_power), jnp.log1p(interference)], -1)
    combined = jnp.concatenate(
        [f_init, f_demod, f_glob, f_chan, bitwidth_features, ifeat, local_snr], -1
    )
    node = mlp2(params["fusion"], combined)
    flat = node.reshape(-1, D)
    mu_loc = flat.mean(0)
    m2_loc = (flat * flat).mean(0)
    mu = jax.lax.pmean(mu_loc, axis_name="x")
    m2 = jax.lax.pmean(m2_loc, axis_name="x")
    var = m2 - mu * mu
    flat = (flat - mu) / jnp.sqrt(var + 1e-5) * params["bn"]["g"] + params["bn"]["b"]
    h = flat.reshape(node.shape)
    for gp in params["gnn"]:
        msg = mlp2(gp["msg"], h)
        mean_msg = jnp.broadcast_to(msg.mean(axis=1, keepdims=True), h.shape)
        upd = mlp2(gp["upd"], jnp.concatenate([h, mean_msg], -1))
        h = layer_norm(h + upd, gp["lng"], gp["lnb"])
    attn = jax.nn.softmax(mlp2(params["agg"], h), axis=1)
    user = (h * attn).sum(axis=1)
    for tp in params["tf"]:
        user = tf_layer(tp, user)
    residual = user @ params["out"]["W"] + params["out"]["b"]
    detected = s_global + params["alpha"] * residual
    return detected, s_global


_pmapped = None
_unpack_pm = None
_dev_params = None
_param_key = None


def kernel(s_hat_q, H_q, y_q, bitwidth_features, local_snr, params):
    global _pmapped, _unpack_pm, _dev_params, _param_key
    devices = jax.devices()[:NCORES]

    # pack all inputs into one per-device blob -> single sharded transfer
    parts = [
        np.ascontiguousarray(np.asarray(a, np.float32)).reshape(NCORES, -1)
        for a in (s_hat_q, H_q, y_q, bitwidth_features, local_snr)
    ]
    blob = np.concatenate(parts, axis=1)
    dblob = jax.device_put_sharded([blob[i] for i in range(NCORES)], devices)

    key = id(params)
    if _dev_params is None or _param_key != key:
        p32 = jax.tree_util.tree_map(
            lambda x: np.asarray(x, np.float32), params
        )
        _dev_params = jax.device_put_replicated(p32, devices)
        _param_key = key

    if _unpack_pm is None:
        _unpack_pm = jax.pmap(_unpack, in_axes=0)
    if _pmapped is None:
        _pmapped = jax.pmap(
            _forward, axis_name="x", in_axes=(0, 0, 0, 0, 0, 0)
        )

    tensors = _unpack_pm(dblob)
    out = _pmapped(*tensors, _dev_params)
    detected, s_global = jax.device_get(out)
    detected = np.asarray(detected, dtype=np.float32).reshape(B, K, 2)
    s_global = np.asarray(s_global, dtype=np.float32).reshape(B, K, 2)
    return detected, s_global
